# revision 22
# baseline (speedup 1.0000x reference)
"""Causal multi-head attention with RoPE on 8 Trainium2 NeuronCores.

Sharding: core = (batch b, head-group hg): b = core//4, hg = core%4.
Each core computes 4 heads of one batch element end-to-end (QKV projection,
RoPE, causal softmax attention, output-projection partial) and the host sums
the 4 per-head-group partials per batch (the "all-reduce" of the O-proj).

v2 design notes (per-core):
  - All matmul operands are 16-bit: fp16 on the q/k/score/output-proj path
    (11-bit mantissa, guaranteed 1 cycle/row + fast weight load; fp32/fp32r
    lowered to fp32_mode=HIGH 4-cycle matmuls in the v1 trace), bf16 on the
    probability path (exp(scores) up to e^17 overflows fp16; bf16 range is
    safe without max-subtraction).
  - Scores computed transposed per head pair: S^T[k,q] for the even head on
    PE row groups 0-1 (contraction partitions 0-63) and the odd head on row
    groups 2-3 concurrently (tile_position row tiling) -> 2x score rate.
  - Causal trimming: per q-chunk of 512, k-tiles past the diagonal are
    skipped; on the 4 diagonal k-tiles the score matmul / exp / AV matmul
    are trimmed to the valid q range, and a single [128,128] triangular
    bf16 mask handles the in-tile triangle.
  - Software-pipelined emission: scores(kt+1) issue before AV(kt) so the PE
    never head-of-line blocks on the ACT-engine exp; projections for chunk
    qc+2 and output-projection of chunk qc-1 are spliced into the attention
    k-loop as PE filler while ACT drains the exp backlog.
  - Softmax denominators ride as a 65th ones-column on the AV matmul; the
    per-q reciprocal broadcast runs on the otherwise-idle GpSimd engine
    (partition_broadcast) in fp32, avoiding both PE broadcast matmuls and
    fp16 range issues on the reciprocal.
"""

from collections import deque

import numpy as np

_B, _L, _D, _H, _HD = 2, 2048, 1024, 16, 64
_HPG = 4              # heads per group (per core)
_EG = _HPG * _HD      # 256
_NCORES = 8
_THETA = 10000.0
_QC = 512             # q-chunk width
_NQC = _L // _QC      # 4
_NKC = _D // 128      # 8 contraction chunks for projections
_LC = 512             # l-chunk for projections
_NLC = _L // _LC

_CACHE = {}


def _build_nc():
    from contextlib import ExitStack

    import concourse.mybir as mybir
    import concourse.tile as tile
    from concourse import bacc

    f32 = mybir.dt.float32
    f16 = mybir.dt.float16
    bf16 = mybir.dt.bfloat16
    EXP = mybir.ActivationFunctionType.Exp

    nc = bacc.Bacc("TRN2", target_bir_lowering=False, debug=False,
                   enable_asserts=False)
    xT = nc.dram_tensor("xT", [_D, _L], f16, kind="ExternalInput")
    wq = nc.dram_tensor("wq", [_D, _EG], f16, kind="ExternalInput")
    wk = nc.dram_tensor("wk", [_D, _EG], f16, kind="ExternalInput")
    wv = nc.dram_tensor("wv", [_D, _EG], f16, kind="ExternalInput")
    wo = nc.dram_tensor("wo", [_EG, _D], f16, kind="ExternalInput")
    cs = nc.dram_tensor("cs", [128, _L], f16, kind="ExternalInput")
    sn = nc.dram_tensor("sn", [128, _L], f16, kind="ExternalInput")
    tri = nc.dram_tensor("tri", [128, 128], bf16, kind="ExternalInput")
    perm = nc.dram_tensor("perm", [128, 128], f16, kind="ExternalInput")
    vones = nc.dram_tensor("vones", [128, _HD], bf16, kind="ExternalInput")
    y = nc.dram_tensor("y", [_L, _D], f16, kind="ExternalOutput")

    with tile.TileContext(nc) as tc, ExitStack() as ctx:
        persist = ctx.enter_context(tc.tile_pool(name="persist", bufs=1))
        xt_sb = persist.tile([128, _NKC, _L], f16)
        qT_sb = persist.tile([128, 2, _L], f16)
        kT_sb = persist.tile([128, 2, _L], f16)
        v_sb = persist.tile([128, _L // 128, _HPG, _HD + 4], bf16)
        oT_sb = persist.tile([128, 2, _L], f16)
        wo_sb = persist.tile([128, 2, _D], f16)
        tri_sb = persist.tile([128, 128], bf16)
        wq_sb = persist.tile([128, _NKC, _EG], f16)
        wk_sb = persist.tile([128, _NKC, _EG], f16)
        wv_sb = persist.tile([128, _NKC, _EG], f16)
        cs_sb = persist.tile([128, _L], f16)
        sn_sb = persist.tile([128, _L], f16)
        perm_sb = persist.tile([128, 128], f16)

        rtmp = ctx.enter_context(tc.tile_pool(name="rtmp", bufs=2))
        ptp = ctx.enter_context(tc.tile_pool(name="ptp", bufs=6))
        nrm = ctx.enter_context(tc.tile_pool(name="nrm", bufs=8))
        otcp = ctx.enter_context(tc.tile_pool(name="otc", bufs=5))
        otnp = ctx.enter_context(tc.tile_pool(name="otn", bufs=4))
        bchp = ctx.enter_context(tc.tile_pool(name="bch", bufs=2))
        obp = ctx.enter_context(tc.tile_pool(name="obp", bufs=3))
        # PSUM budget (8 banks): sps 4x1 + ops 2x1 + scr 2x1
        sps = ctx.enter_context(tc.tile_pool(name="sps", bufs=2, space="PSUM"))
        ops = ctx.enter_context(tc.tile_pool(name="ops", bufs=2, space="PSUM"))
        scr = ctx.enter_context(tc.tile_pool(name="scr", bufs=2, space="PSUM"))

        # --- input loads, priority order: first-chunk operands first so
        # compute starts as soon as the first slices land ---
        wq_r = wq.rearrange("(c p) e -> p c e", p=128)
        wk_r = wk.rearrange("(c p) e -> p c e", p=128)
        wv_r = wv.rearrange("(c p) e -> p c e", p=128)
        xT_r = xT.rearrange("(c p) l -> p c l", p=128)
        # spread the big input DMAs round-robin over 5 engine queues to
        # raise aggregate HBM read bandwidth during the startup window
        qs_ = (nc.sync, nc.scalar)
        qi = [0]

        def ddma(out, in_):
            qs_[qi[0] % len(qs_)].dma_start(out=out, in_=in_)
            qi[0] += 1

        nc.gpsimd.dma_start(out=perm_sb, in_=perm[:, :])
        for lc in (0, 1):
            ls = slice(lc * _LC, (lc + 1) * _LC)
            nc.gpsimd.dma_start(out=cs_sb[:, ls], in_=cs[:, ls])
            nc.gpsimd.dma_start(out=sn_sb[:, ls], in_=sn[:, ls])
        for kc in range(_NKC):
            ddma(wq_sb[:, kc, :], wq_r[:, kc, :])
            ddma(xt_sb[:, kc, 0:_LC], xT_r[:, kc, 0:_LC])
            ddma(wk_sb[:, kc, :], wk_r[:, kc, :])
            ddma(wv_sb[:, kc, :], wv_r[:, kc, :])
        for kc in range(_NKC):
            ddma(xt_sb[:, kc, _LC:2 * _LC], xT_r[:, kc, _LC:2 * _LC])
        nc.gpsimd.dma_start(out=tri_sb, in_=tri[:, :])
        nc.gpsimd.dma_start(
            out=v_sb[:, :, :, _HD:_HD + 1],
            in_=vones.rearrange("p (a b) -> p a b",
                                a=_L // 128).unsqueeze(3))
        for lc in (2, 3):
            ls = slice(lc * _LC, (lc + 1) * _LC)
            nc.gpsimd.dma_start(out=cs_sb[:, ls], in_=cs[:, ls])
            nc.gpsimd.dma_start(out=sn_sb[:, ls], in_=sn[:, ls])
        for lc in range(2, _NLC):
            for kc in range(_NKC):
                ddma(xt_sb[:, kc, lc * _LC:(lc + 1) * _LC],
                     xT_r[:, kc, lc * _LC:(lc + 1) * _LC])
        nc.gpsimd.dma_start(out=wo_sb,
                            in_=wo.rearrange("(c p) d -> p c d", p=128))

        # HAM warmup: a burst of throwaway matmuls on zeros keeps the PE
        # activity monitor busy during the input-DMA window so real compute
        # starts at full clock (2.4 GHz) instead of the cold 1.2 GHz
        wup = rtmp.tile([128, _LC], f16, tag="rt", name="wup")
        nc.vector.memset(wup, 0.0)
        wps = scr.tile([128, _LC], f32, tag="scr", name="wps")
        for i in range(24):
            nc.tensor.matmul(wps, wup[:, 0:128], wup,
                             start=(i == 0), stop=(i == 23))

        # ---------- emission helpers ----------

        def qk_proj_unit(lc, w_sb, dst, c):
            """one [128, LC] projection accumulation + copy to f16."""
            ls = slice(lc * _LC, (lc + 1) * _LC)
            ps = scr.tile([128, _LC], f32, tag="scr",
                          name=f"ps{lc}_{id(dst)}_{c}")
            for kc in range(_NKC):
                nc.tensor.matmul(
                    ps, w_sb[:, kc, c * 128:(c + 1) * 128],
                    xt_sb[:, kc, ls],
                    start=(kc == 0), stop=(kc == _NKC - 1))
            nc.vector.tensor_copy(dst[:, c, ls], ps)

        def rope_unit(lc, dst, c):
            """in-place RoPE on dst[:, c, lc-chunk]: x*cos + (P@x)*sin."""
            ls = slice(lc * _LC, (lc + 1) * _LC)
            rp = scr.tile([128, _LC], f32, tag="scr", name=f"rp{lc}_{c}")
            nc.tensor.matmul(rp, perm_sb[:, :], dst[:, c, ls],
                             start=True, stop=True)
            tmp = rtmp.tile([128, _LC], f16, tag="rt")
            nc.vector.tensor_mul(tmp, rp, sn_sb[:, ls])
            nc.vector.tensor_mul(dst[:, c, ls], dst[:, c, ls], cs_sb[:, ls])
            nc.vector.tensor_add(dst[:, c, ls], dst[:, c, ls], tmp)

        def v_proj_unit(lc, j):
            """V projection for l-tile j of chunk lc -> v_sb (bf16)."""
            lt = lc * (_LC // 128) + j
            pv = scr.tile([128, _EG], f32, tag="scr", name=f"pv{lt}")
            for kc in range(_NKC):
                nc.tensor.matmul(
                    pv, xt_sb[:, kc, lt * 128:(lt + 1) * 128],
                    wv_sb[:, kc, :],
                    start=(kc == 0), stop=(kc == _NKC - 1))
            nc.vector.tensor_copy(
                v_sb[:, lt, :, :_HD],
                pv.rearrange("p (h e) -> p h e", h=_HPG))

        def proj_chunk_units(lc):
            units = []
            for dst, w_sb in ((qT_sb, wq_sb), (kT_sb, wk_sb)):
                for c in range(2):
                    units.append(lambda lc=lc, w=w_sb, d=dst, c=c:
                                 qk_proj_unit(lc, w, d, c))
                    units.append(lambda lc=lc, d=dst, c=c:
                                 rope_unit(lc, d, c))
            for j in range(_LC // 128):
                units.append(lambda lc=lc, j=j: v_proj_unit(lc, j))
            return units

        def oproj_unit(qc, j, pool=None):
            """output projection for l-tile j of chunk qc (cc-outer so the
            oT stationary is loaded once per contraction half)."""
            lt = qc * (_QC // 128) + j
            if pool is None:
                opn = [scr.tile([128, 512], f32, tag="scr",
                                name=f"op{qc}_{j}_{n}") for n in range(2)]
            else:
                opt = pool.tile([128, 2, _QC], f32, tag="sp",
                                name=f"op{qc}_{j}")
                opn = [opt[:, 0, :], opt[:, 1, :]]
            for cc in range(2):
                for n in range(2):
                    nc.tensor.matmul(
                        opn[n], oT_sb[:, cc, lt * 128:(lt + 1) * 128],
                        wo_sb[:, cc, n * 512:(n + 1) * 512],
                        start=(cc == 0), stop=(cc == 1),
                        skip_group_check=True)
            for n in range(2):
                ob = obp.tile([128, 512], f16, tag="ob")
                nc.vector.tensor_copy(ob, opn[n])
                nc.sync.dma_start(
                    out=y[lt * 128:(lt + 1) * 128, n * 512:(n + 1) * 512],
                    in_=ob)

        filler = deque()

        def drain_filler(slots_left):
            n = len(filler)
            if n == 0 or slots_left <= 0:
                return
            take = (n + slots_left - 1) // slots_left
            for _ in range(min(take, n)):
                filler.popleft()()

        def av_pair(qc, c, kt, nkt, ot, pt):
            po = 128 * (kt - qc * (_QC // 128))
            po = po if po > 0 else 0
            for t, pb in ((0, 0), (1, 64)):
                nc.tensor.matmul(
                    ot[pb][:, po:], v_sb[:, kt, 2 * c + t, :_HD + 1],
                    pt[:, t, po:],
                    start=(kt == 0), stop=(kt == nkt - 1),
                    skip_group_check=True)

        def attn_chunk(qc, mid_units=()):
            """attention for q-chunk qc; drains filler into PE gaps.
            mid_units join the filler at the second pair (late enough that
            their dependencies from the first pair's deferred work are met).
            """
            q0 = qc * _QC
            nkt = (qc + 1) * (_QC // 128)
            slots = 2 * nkt + 2
            for c in range(2):            # head pair (2c, 2c+1)
                if c == 1:
                    filler.extend(mid_units)
                ot = {}
                for pb in (0, 64):
                    h = 2 * c + pb // 64
                    ot[pb] = ops.tile([_HD + 1, _QC], f32, tag="ot",
                                      name=f"ot{qc}_{h}")
                pts = {}
                for kt in range(nkt):
                    dj = kt - qc * (_QC // 128)
                    off = 128 * dj if dj >= 0 else 0
                    qs = slice(q0 + off, q0 + _QC)
                    ks = slice(kt * 128, (kt + 1) * 128)
                    # concurrent row-tiled score matmuls (even head on PE
                    # rows 0-63, odd head on rows 64-127) into one 2-bank
                    # PSUM tile, exp'd by a single ACT instruction
                    spt = sps.tile([128, 2, _QC], f32, tag="sp",
                                   name=f"sp{qc}_{c}_{kt}")
                    for t, pb in ((0, 0), (1, 64)):
                        nc.tensor.matmul(
                            spt[:, t, off:], kT_sb[pb:pb + 64, c, ks],
                            qT_sb[pb:pb + 64, c, qs],
                            start=True, stop=True)
                    pt = ptp.tile([128, 2, _QC], bf16, tag="pt",
                                  name=f"pt{qc}_{c}_{kt}")
                    nc.scalar.activation(pt[:, :, off:], spt[:, :, off:],
                                         EXP, scale=0.125)
                    if dj >= 0:
                        for t in range(2):
                            nc.vector.tensor_mul(
                                pt[:, t, off:off + 128],
                                pt[:, t, off:off + 128], tri_sb)
                    pts[kt] = pt
                    drain_filler(slots - 1)
                    slots -= 1
                    if kt > 0:
                        av_pair(qc, c, kt - 1, nkt, ot, pts.pop(kt - 1))
                av_pair(qc, c, nkt - 1, nkt, ot, pts.pop(nkt - 1))
                # per-pair normalization: drain PSUM to f32 SBUF (den must
                # stay f32: up to ~5e10), reciprocal of the den row in
                # place, gpsimd partition-broadcast, multiply, place into
                # oT. Deferred into the next pair's k-loop as filler so the
                # serial cross-engine chain overlaps matmul streams.
                otcs = {}

                def otc_unit(t, pb, qc=qc, c=c, ot=ot, otcs=otcs):
                    h = 2 * c + t
                    otc = otcp.tile([_HD + 1, _QC], f32, tag="otc",
                                    name=f"otc{qc}_{h}")
                    nc.vector.tensor_copy(otc, ot[pb][:, :])
                    dsb = nrm.tile([128, 4], f32, tag="dsb",
                                   name=f"dsb{qc}_{h}")
                    nc.gpsimd.dma_start(out=dsb, in_=otc[64:65, :])
                    otcs[h] = (otc, dsb)

                def fin_unit(t, pb, qc=qc, c=c, q0=q0, otcs=otcs):
                    # after the reciprocal, the whole chain stays on the
                    # gpsimd queue so its serial latency never head-of-line
                    # blocks the DVE queue (which feeds RoPE/projections);
                    # the final pair uses the then-idle DVE/sync engines
                    last = (qc == _NQC - 1 and c == 1)
                    h = 2 * c + t
                    otc, dsb = otcs[h]
                    drc = nrm.tile([128, 4], f32, tag="drc",
                                   name=f"drc{qc}_{h}")
                    nc.vector.reciprocal(drc, dsb)
                    drw = nrm.tile([1, _QC], f32, tag="drw",
                                   name=f"drw{qc}_{h}")
                    nc.gpsimd.dma_start(out=drw[0:1, :], in_=drc)
                    bch = bchp.tile([_HD, _QC], f32, tag="bch",
                                    name=f"bch{qc}_{h}")
                    nc.gpsimd.partition_broadcast(bch, drw[0:1, :],
                                                  channels=_HD)
                    otn = otnp.tile([_HD, _QC], f16, tag="otn",
                                    name=f"otn{qc}_{h}")
                    eng = nc.vector if last else nc.gpsimd
                    eng.tensor_mul(otn, otc[0:_HD, :], bch)
                    dq = nc.sync if last else nc.gpsimd
                    dq.dma_start(
                        out=oT_sb[pb:pb + 64, c, q0:q0 + _QC], in_=otn)

                units = [lambda: otc_unit(0, 0), lambda: otc_unit(1, 64),
                         lambda: fin_unit(0, 0), lambda: fin_unit(1, 64)]
                if qc == _NQC - 1 and c == 1:
                    for u in units:
                        u()
                else:
                    filler.extendleft(reversed(units))
                drain_filler(slots - 1)
                slots -= 1

        # ---------- program ----------
        for u in proj_chunk_units(0):
            u()
        for u in proj_chunk_units(1):
            u()
        for qc in range(_NQC):
            if qc + 2 < _NLC:
                filler.extend(proj_chunk_units(qc + 2))
            mid = ()
            if qc >= 1:
                mid = [lambda qc=qc, j=j: oproj_unit(qc - 1, j)
                       for j in range(_QC // 128)]
            attn_chunk(qc, mid_units=mid)
        while filler:
            filler.popleft()()
        for j in range(_QC // 128):
            oproj_unit(_NQC - 1, j, pool=sps)
    nc.compile()
    return nc


def get_nc():
    if "nc" not in _CACHE:
        _CACHE["nc"] = _build_nc()
    return _CACHE["nc"]


def make_in_maps(x, token_positions, Q, K, V, O_w):
    """Host-side sharding: per-core input dict (core = b*4 + hg)."""
    import ml_dtypes
    bf16 = ml_dtypes.bfloat16
    f16 = np.float16
    x = np.asarray(x, dtype=np.float32)
    tp = np.asarray(token_positions)
    Q = np.asarray(Q, dtype=np.float32)
    K = np.asarray(K, dtype=np.float32)
    V = np.asarray(V, dtype=np.float32)
    O_w = np.asarray(O_w, dtype=np.float32)

    # RoPE tables, [128, L]: rows 0..63 head-local e (cos repeated pairwise),
    # rows 64..127 a copy (two heads share one partition tile).
    i = np.arange(_HD // 2, dtype=np.float64)
    denom = _THETA ** (2.0 * i / _HD)                      # [32]
    ang = tp.astype(np.float64)[None, :] / denom[:, None]  # [32, L]
    cs64 = np.repeat(np.cos(ang), 2, axis=0)
    sn64 = np.repeat(np.sin(ang), 2, axis=0)
    cs = np.vstack([cs64, cs64]).astype(f16)
    sn = np.vstack([sn64, sn64]).astype(f16)

    # pairwise-rotation permutation (rot(x)[2i] = -x[2i+1], rot[2i+1] = x[2i])
    # as a stationary operand: out = permT.T @ x^T = Perm @ x^T
    p64 = np.zeros((64, 64), np.float32)
    for j in range(_HD // 2):
        p64[2 * j + 1, 2 * j] = -1.0
        p64[2 * j, 2 * j + 1] = 1.0
    permT = np.zeros((128, 128), f16)
    permT[0:64, 0:64] = p64
    permT[64:128, 64:128] = p64

    # triangular mask for the in-tile diagonal: valid iff q_local >= p
    pp = np.arange(128)[:, None]
    qq = np.arange(128)[None, :]
    tri = (qq >= pp).astype(bf16)                          # [128, 128]

    Qr = Q.reshape(_H, _HD, _D)
    Kr = K.reshape(_H, _HD, _D)
    Vr = V.reshape(_H, _HD, _D)

    in_maps = []
    xT = [np.ascontiguousarray(x[b].T).astype(f16) for b in range(_B)]
    for core in range(_NCORES):
        b, hg = core // 4, core % 4
        hs = slice(hg * _HPG, (hg + 1) * _HPG)
        in_maps.append({
            "xT": xT[b],
            "wq": Qr[hs].reshape(_EG, _D).T.astype(f16),
            "wk": Kr[hs].reshape(_EG, _D).T.astype(f16),
            "wv": Vr[hs].reshape(_EG, _D).T.astype(f16),
            "wo": O_w[:, hg * _EG:(hg + 1) * _EG].T.astype(f16),
            "cs": cs, "sn": sn, "tri": tri, "perm": permT,
            "vones": np.ones((128, _HD), bf16),
        })
    return in_maps


def run_on_hw(in_maps, trace=False, **kw):
    from concourse.bass_utils import run_bass_kernel_spmd
    nc = get_nc()
    return run_bass_kernel_spmd(nc, in_maps, core_ids=list(range(_NCORES)),
                                trace=trace, **kw)


def kernel(x, token_positions, Q, K, V, O_w):
    in_maps = make_in_maps(x, token_positions, Q, K, V, O_w)
    res = run_on_hw(in_maps)
    out = np.zeros((_B, _L, _D), dtype=np.float32)
    for core in range(_NCORES):
        out[core // 4] += np.asarray(res.results[core]["y"],
                                     dtype=np.float32)
    return out


# revision 23
# speedup vs baseline: 1.8372x; 1.8372x over previous
"""Causal multi-head attention with RoPE on 8 Trainium2 NeuronCores.

Sharding: core = (batch b, head-group hg): b = core//4, hg = core%4.
Each core computes 4 heads of one batch element end-to-end (QKV projection,
RoPE, causal softmax attention, output-projection partial) and the host sums
the 4 per-head-group partials per batch (the "all-reduce" of the O-proj).

v2 design notes (per-core):
  - All matmul operands are 16-bit: fp16 on the q/k/score/output-proj path
    (11-bit mantissa, guaranteed 1 cycle/row + fast weight load; fp32/fp32r
    lowered to fp32_mode=HIGH 4-cycle matmuls in the v1 trace), bf16 on the
    probability path (exp(scores) up to e^17 overflows fp16; bf16 range is
    safe without max-subtraction).
  - Scores computed transposed per head pair: S^T[k,q] for the even head on
    PE row groups 0-1 (contraction partitions 0-63) and the odd head on row
    groups 2-3 concurrently (tile_position row tiling) -> 2x score rate.
  - Causal trimming: per q-chunk of 512, k-tiles past the diagonal are
    skipped; on the 4 diagonal k-tiles the score matmul / exp / AV matmul
    are trimmed to the valid q range, and a single [128,128] triangular
    bf16 mask handles the in-tile triangle.
  - Software-pipelined emission: scores(kt+1) issue before AV(kt) so the PE
    never head-of-line blocks on the ACT-engine exp; projections for chunk
    qc+2 and output-projection of chunk qc-1 are spliced into the attention
    k-loop as PE filler while ACT drains the exp backlog.
  - Softmax denominators ride as a 65th ones-column on the AV matmul; the
    per-q reciprocal broadcast runs on the otherwise-idle GpSimd engine
    (partition_broadcast) in fp32, avoiding both PE broadcast matmuls and
    fp16 range issues on the reciprocal.
"""

from collections import deque

import numpy as np

_B, _L, _D, _H, _HD = 2, 2048, 1024, 16, 64
_HPG = 4              # heads per group (per core)
_EG = _HPG * _HD      # 256
_NCORES = 8
_THETA = 10000.0
_QC = 512             # q-chunk width
_NQC = _L // _QC      # 4
_NKC = _D // 128      # 8 contraction chunks for projections
_LC = 512             # l-chunk for projections
_NLC = _L // _LC

_CACHE = {}


def _build_nc():
    from contextlib import ExitStack

    import concourse.mybir as mybir
    import concourse.tile as tile
    from concourse import bacc

    f32 = mybir.dt.float32
    f16 = mybir.dt.float16
    bf16 = mybir.dt.bfloat16
    EXP = mybir.ActivationFunctionType.Exp

    nc = bacc.Bacc("TRN2", target_bir_lowering=False, debug=False,
                   enable_asserts=False)
    xT = nc.dram_tensor("xT", [_D, _L], f16, kind="ExternalInput")
    wq = nc.dram_tensor("wq", [_D, _EG], f16, kind="ExternalInput")
    wk = nc.dram_tensor("wk", [_D, _EG], f16, kind="ExternalInput")
    wv = nc.dram_tensor("wv", [_D, _EG], f16, kind="ExternalInput")
    wo = nc.dram_tensor("wo", [_EG, _D], f16, kind="ExternalInput")
    cs = nc.dram_tensor("cs", [128, _L], f16, kind="ExternalInput")
    sn = nc.dram_tensor("sn", [128, _L], f16, kind="ExternalInput")
    tri = nc.dram_tensor("tri", [128, 128], bf16, kind="ExternalInput")
    perm = nc.dram_tensor("perm", [128, 128], f16, kind="ExternalInput")
    vones = nc.dram_tensor("vones", [128, _HD], bf16, kind="ExternalInput")
    y = nc.dram_tensor("y", [_L, _D], f16, kind="ExternalOutput")

    with tile.TileContext(nc) as tc, ExitStack() as ctx:
        persist = ctx.enter_context(tc.tile_pool(name="persist", bufs=1))
        xt_sb = persist.tile([128, _NKC, _L], f16)
        qT_sb = persist.tile([128, 2, _L], f16)
        kT_sb = persist.tile([128, 2, _L], f16)
        v_sb = persist.tile([128, _L // 128, _HPG, _HD + 4], bf16)
        oT_sb = persist.tile([128, 2, _L], f16)
        wo_sb = persist.tile([128, 2, _D], f16)
        tri_sb = persist.tile([128, 128], bf16)
        wq_sb = persist.tile([128, _NKC, _EG], f16)
        wk_sb = persist.tile([128, _NKC, _EG], f16)
        wv_sb = persist.tile([128, _NKC, _EG], f16)
        cs_sb = persist.tile([128, _L], f16)
        sn_sb = persist.tile([128, _L], f16)
        perm_sb = persist.tile([128, 128], f16)

        rtmp = ctx.enter_context(tc.tile_pool(name="rtmp", bufs=2))
        ptp = ctx.enter_context(tc.tile_pool(name="ptp", bufs=6))
        nrm = ctx.enter_context(tc.tile_pool(name="nrm", bufs=8))
        otcp = ctx.enter_context(tc.tile_pool(name="otc", bufs=5))
        otnp = ctx.enter_context(tc.tile_pool(name="otn", bufs=4))
        bchp = ctx.enter_context(tc.tile_pool(name="bch", bufs=2))
        obp = ctx.enter_context(tc.tile_pool(name="obp", bufs=3))
        # PSUM budget (8 banks): sps 4x1 + ops 2x1 + scr 2x1
        sps = ctx.enter_context(tc.tile_pool(name="sps", bufs=2, space="PSUM"))
        ops = ctx.enter_context(tc.tile_pool(name="ops", bufs=2, space="PSUM"))
        scr = ctx.enter_context(tc.tile_pool(name="scr", bufs=2, space="PSUM"))

        # --- input loads, priority order: first-chunk operands first so
        # compute starts as soon as the first slices land ---
        wq_r = wq.rearrange("(c p) e -> p c e", p=128)
        wk_r = wk.rearrange("(c p) e -> p c e", p=128)
        wv_r = wv.rearrange("(c p) e -> p c e", p=128)
        xT_r = xT.rearrange("(c p) l -> p c l", p=128)
        # spread the big input DMAs round-robin over 5 engine queues to
        # raise aggregate HBM read bandwidth during the startup window
        qs_ = (nc.sync, nc.scalar)
        qi = [0]

        def ddma(out, in_):
            qs_[qi[0] % len(qs_)].dma_start(out=out, in_=in_)
            qi[0] += 1

        nc.gpsimd.dma_start(out=perm_sb, in_=perm[:, :])
        for lc in (0, 1):
            ls = slice(lc * _LC, (lc + 1) * _LC)
            nc.gpsimd.dma_start(out=cs_sb[:, ls], in_=cs[:, ls])
            nc.gpsimd.dma_start(out=sn_sb[:, ls], in_=sn[:, ls])
        for kc in range(_NKC):
            ddma(wq_sb[:, kc, :], wq_r[:, kc, :])
            ddma(xt_sb[:, kc, 0:_LC], xT_r[:, kc, 0:_LC])
            ddma(wk_sb[:, kc, :], wk_r[:, kc, :])
            ddma(wv_sb[:, kc, :], wv_r[:, kc, :])
        for kc in range(_NKC):
            ddma(xt_sb[:, kc, _LC:2 * _LC], xT_r[:, kc, _LC:2 * _LC])
        nc.gpsimd.dma_start(out=tri_sb, in_=tri[:, :])
        nc.gpsimd.dma_start(
            out=v_sb[:, :, :, _HD:_HD + 1],
            in_=vones.rearrange("p (a b) -> p a b",
                                a=_L // 128).unsqueeze(3))
        for lc in (2, 3):
            ls = slice(lc * _LC, (lc + 1) * _LC)
            nc.gpsimd.dma_start(out=cs_sb[:, ls], in_=cs[:, ls])
            nc.gpsimd.dma_start(out=sn_sb[:, ls], in_=sn[:, ls])
        for lc in range(2, _NLC):
            for kc in range(_NKC):
                ddma(xt_sb[:, kc, lc * _LC:(lc + 1) * _LC],
                     xT_r[:, kc, lc * _LC:(lc + 1) * _LC])
        nc.gpsimd.dma_start(out=wo_sb,
                            in_=wo.rearrange("(c p) d -> p c d", p=128))

        # HAM warmup: a burst of throwaway matmuls on zeros keeps the PE
        # activity monitor busy during the input-DMA window so real compute
        # starts at full clock (2.4 GHz) instead of the cold 1.2 GHz
        wup = rtmp.tile([128, _LC], f16, tag="rt", name="wup")
        nc.vector.memset(wup, 0.0)
        wps = scr.tile([128, _LC], f32, tag="scr", name="wps")
        for i in range(24):
            nc.tensor.matmul(wps, wup[:, 0:128], wup,
                             start=(i == 0), stop=(i == 23))

        # ---------- emission helpers ----------

        def qk_proj_unit(lc, w_sb, dst, c):
            """one [128, LC] projection accumulation + copy to f16."""
            ls = slice(lc * _LC, (lc + 1) * _LC)
            ps = scr.tile([128, _LC], f32, tag="scr",
                          name=f"ps{lc}_{id(dst)}_{c}")
            for kc in range(_NKC):
                nc.tensor.matmul(
                    ps, w_sb[:, kc, c * 128:(c + 1) * 128],
                    xt_sb[:, kc, ls],
                    start=(kc == 0), stop=(kc == _NKC - 1))
            nc.vector.tensor_copy(dst[:, c, ls], ps)

        def rope_unit(lc, dst, c):
            """in-place RoPE on dst[:, c, lc-chunk]: x*cos + (P@x)*sin."""
            ls = slice(lc * _LC, (lc + 1) * _LC)
            rp = scr.tile([128, _LC], f32, tag="scr", name=f"rp{lc}_{c}")
            nc.tensor.matmul(rp, perm_sb[:, :], dst[:, c, ls],
                             start=True, stop=True)
            tmp = rtmp.tile([128, _LC], f16, tag="rt")
            nc.vector.tensor_mul(tmp, rp, sn_sb[:, ls])
            nc.vector.tensor_mul(dst[:, c, ls], dst[:, c, ls], cs_sb[:, ls])
            nc.vector.tensor_add(dst[:, c, ls], dst[:, c, ls], tmp)

        def v_proj_unit(lc, j):
            """V projection for l-tile j of chunk lc -> v_sb (bf16)."""
            lt = lc * (_LC // 128) + j
            pv = scr.tile([128, _EG], f32, tag="scr", name=f"pv{lt}")
            for kc in range(_NKC):
                nc.tensor.matmul(
                    pv, xt_sb[:, kc, lt * 128:(lt + 1) * 128],
                    wv_sb[:, kc, :],
                    start=(kc == 0), stop=(kc == _NKC - 1))
            nc.vector.tensor_copy(
                v_sb[:, lt, :, :_HD],
                pv.rearrange("p (h e) -> p h e", h=_HPG))

        def proj_chunk_units(lc):
            units = []
            for dst, w_sb in ((qT_sb, wq_sb), (kT_sb, wk_sb)):
                for c in range(2):
                    units.append(lambda lc=lc, w=w_sb, d=dst, c=c:
                                 qk_proj_unit(lc, w, d, c))
                    units.append(lambda lc=lc, d=dst, c=c:
                                 rope_unit(lc, d, c))
            for j in range(_LC // 128):
                units.append(lambda lc=lc, j=j: v_proj_unit(lc, j))
            return units

        def oproj_unit(qc, j, pool=None):
            """output projection for l-tile j of chunk qc (cc-outer so the
            oT stationary is loaded once per contraction half)."""
            lt = qc * (_QC // 128) + j
            if pool is None:
                opn = [scr.tile([128, 512], f32, tag="scr",
                                name=f"op{qc}_{j}_{n}") for n in range(2)]
            else:
                opt = pool.tile([128, 2, _QC], f32, tag="sp",
                                name=f"op{qc}_{j}")
                opn = [opt[:, 0, :], opt[:, 1, :]]
            for cc in range(2):
                for n in range(2):
                    nc.tensor.matmul(
                        opn[n], oT_sb[:, cc, lt * 128:(lt + 1) * 128],
                        wo_sb[:, cc, n * 512:(n + 1) * 512],
                        start=(cc == 0), stop=(cc == 1),
                        skip_group_check=True)
            for n in range(2):
                ob = obp.tile([128, 512], f16, tag="ob")
                nc.vector.tensor_copy(ob, opn[n])
                nc.sync.dma_start(
                    out=y[lt * 128:(lt + 1) * 128, n * 512:(n + 1) * 512],
                    in_=ob)

        filler = deque()

        def drain_filler(slots_left):
            n = len(filler)
            if n == 0 or slots_left <= 0:
                return
            take = (n + slots_left - 1) // slots_left
            for _ in range(min(take, n)):
                filler.popleft()()

        def av_pair(qc, c, kt, nkt, ot, pt):
            po = 128 * (kt - qc * (_QC // 128))
            po = po if po > 0 else 0
            for t, pb in ((0, 0), (1, 64)):
                nc.tensor.matmul(
                    ot[pb][:, po:], v_sb[:, kt, 2 * c + t, :_HD + 1],
                    pt[:, t, po:],
                    start=(kt == 0), stop=(kt == nkt - 1),
                    skip_group_check=True)

        def attn_chunk(qc, mid_units=()):
            """attention for q-chunk qc; drains filler into PE gaps.
            mid_units join the filler at the second pair (late enough that
            their dependencies from the first pair's deferred work are met).
            """
            q0 = qc * _QC
            nkt = (qc + 1) * (_QC // 128)
            slots = 2 * nkt + 2
            for c in range(2):            # head pair (2c, 2c+1)
                if c == 1:
                    filler.extend(mid_units)
                ot = {}
                for pb in (0, 64):
                    h = 2 * c + pb // 64
                    ot[pb] = ops.tile([_HD + 1, _QC], f32, tag="ot",
                                      name=f"ot{qc}_{h}")
                pts = {}
                for kt in range(nkt):
                    dj = kt - qc * (_QC // 128)
                    off = 128 * dj if dj >= 0 else 0
                    qs = slice(q0 + off, q0 + _QC)
                    ks = slice(kt * 128, (kt + 1) * 128)
                    # concurrent row-tiled score matmuls (even head on PE
                    # rows 0-63, odd head on rows 64-127) into one 2-bank
                    # PSUM tile, exp'd by a single ACT instruction
                    spt = sps.tile([128, 2, _QC], f32, tag="sp",
                                   name=f"sp{qc}_{c}_{kt}")
                    for t, pb in ((0, 0), (1, 64)):
                        nc.tensor.matmul(
                            spt[:, t, off:], kT_sb[pb:pb + 64, c, ks],
                            qT_sb[pb:pb + 64, c, qs],
                            start=True, stop=True)
                    pt = ptp.tile([128, 2, _QC], bf16, tag="pt",
                                  name=f"pt{qc}_{c}_{kt}")
                    nc.scalar.activation(pt[:, :, off:], spt[:, :, off:],
                                         EXP, scale=0.125)
                    if dj >= 0:
                        for t in range(2):
                            nc.vector.tensor_mul(
                                pt[:, t, off:off + 128],
                                pt[:, t, off:off + 128], tri_sb)
                    pts[kt] = pt
                    drain_filler(slots - 1)
                    slots -= 1
                    if kt > 0:
                        av_pair(qc, c, kt - 1, nkt, ot, pts.pop(kt - 1))
                av_pair(qc, c, nkt - 1, nkt, ot, pts.pop(nkt - 1))
                # per-pair normalization: drain PSUM to f32 SBUF (den must
                # stay f32: up to ~5e10), reciprocal of the den row in
                # place, gpsimd partition-broadcast, multiply, place into
                # oT. Deferred into the next pair's k-loop as filler so the
                # serial cross-engine chain overlaps matmul streams.
                otcs = {}

                def otc_unit(t, pb, qc=qc, c=c, ot=ot, otcs=otcs):
                    h = 2 * c + t
                    otc = otcp.tile([_HD + 1, _QC], f32, tag="otc",
                                    name=f"otc{qc}_{h}")
                    nc.vector.tensor_copy(otc, ot[pb][:, :])
                    dsb = nrm.tile([128, 4], f32, tag="dsb",
                                   name=f"dsb{qc}_{h}")
                    nc.gpsimd.dma_start(out=dsb, in_=otc[64:65, :])
                    otcs[h] = (otc, dsb)

                def fin_unit(t, pb, qc=qc, c=c, q0=q0, otcs=otcs):
                    # after the reciprocal, the whole chain stays on the
                    # gpsimd queue so its serial latency never head-of-line
                    # blocks the DVE queue (which feeds RoPE/projections);
                    # the final pair uses the then-idle DVE/sync engines
                    last = (qc == _NQC - 1 and c == 1)
                    h = 2 * c + t
                    otc, dsb = otcs[h]
                    drc = nrm.tile([128, 4], f32, tag="drc",
                                   name=f"drc{qc}_{h}")
                    nc.vector.reciprocal(drc, dsb)
                    drw = nrm.tile([1, _QC], f32, tag="drw",
                                   name=f"drw{qc}_{h}")
                    nc.gpsimd.dma_start(out=drw[0:1, :], in_=drc)
                    bch = bchp.tile([_HD, _QC], f32, tag="bch",
                                    name=f"bch{qc}_{h}")
                    nc.gpsimd.partition_broadcast(bch, drw[0:1, :],
                                                  channels=_HD)
                    otn = otnp.tile([_HD, _QC], f16, tag="otn",
                                    name=f"otn{qc}_{h}")
                    nc.vector.tensor_mul(otn, otc[0:_HD, :], bch)
                    dq = nc.sync if last else nc.gpsimd
                    dq.dma_start(
                        out=oT_sb[pb:pb + 64, c, q0:q0 + _QC], in_=otn)

                units = [lambda: otc_unit(0, 0), lambda: otc_unit(1, 64),
                         lambda: fin_unit(0, 0), lambda: fin_unit(1, 64)]
                if qc == _NQC - 1 and c == 1:
                    for u in units:
                        u()
                else:
                    filler.extendleft(reversed(units))
                drain_filler(slots - 1)
                slots -= 1

        # ---------- program ----------
        for u in proj_chunk_units(0):
            u()
        for u in proj_chunk_units(1):
            u()
        for qc in range(_NQC):
            if qc + 2 < _NLC:
                filler.extend(proj_chunk_units(qc + 2))
            mid = ()
            if qc >= 1:
                mid = [lambda qc=qc, j=j: oproj_unit(qc - 1, j)
                       for j in range(_QC // 128)]
            attn_chunk(qc, mid_units=mid)
        while filler:
            filler.popleft()()
        for j in range(_QC // 128):
            oproj_unit(_NQC - 1, j, pool=sps)
    nc.compile()
    return nc


def get_nc():
    if "nc" not in _CACHE:
        _CACHE["nc"] = _build_nc()
    return _CACHE["nc"]


def make_in_maps(x, token_positions, Q, K, V, O_w):
    """Host-side sharding: per-core input dict (core = b*4 + hg)."""
    import ml_dtypes
    bf16 = ml_dtypes.bfloat16
    f16 = np.float16
    x = np.asarray(x, dtype=np.float32)
    tp = np.asarray(token_positions)
    Q = np.asarray(Q, dtype=np.float32)
    K = np.asarray(K, dtype=np.float32)
    V = np.asarray(V, dtype=np.float32)
    O_w = np.asarray(O_w, dtype=np.float32)

    # RoPE tables, [128, L]: rows 0..63 head-local e (cos repeated pairwise),
    # rows 64..127 a copy (two heads share one partition tile).
    i = np.arange(_HD // 2, dtype=np.float64)
    denom = _THETA ** (2.0 * i / _HD)                      # [32]
    ang = tp.astype(np.float64)[None, :] / denom[:, None]  # [32, L]
    cs64 = np.repeat(np.cos(ang), 2, axis=0)
    sn64 = np.repeat(np.sin(ang), 2, axis=0)
    cs = np.vstack([cs64, cs64]).astype(f16)
    sn = np.vstack([sn64, sn64]).astype(f16)

    # pairwise-rotation permutation (rot(x)[2i] = -x[2i+1], rot[2i+1] = x[2i])
    # as a stationary operand: out = permT.T @ x^T = Perm @ x^T
    p64 = np.zeros((64, 64), np.float32)
    for j in range(_HD // 2):
        p64[2 * j + 1, 2 * j] = -1.0
        p64[2 * j, 2 * j + 1] = 1.0
    permT = np.zeros((128, 128), f16)
    permT[0:64, 0:64] = p64
    permT[64:128, 64:128] = p64

    # triangular mask for the in-tile diagonal: valid iff q_local >= p
    pp = np.arange(128)[:, None]
    qq = np.arange(128)[None, :]
    tri = (qq >= pp).astype(bf16)                          # [128, 128]

    Qr = Q.reshape(_H, _HD, _D)
    Kr = K.reshape(_H, _HD, _D)
    Vr = V.reshape(_H, _HD, _D)

    in_maps = []
    xT = [np.ascontiguousarray(x[b].T).astype(f16) for b in range(_B)]
    for core in range(_NCORES):
        b, hg = core // 4, core % 4
        hs = slice(hg * _HPG, (hg + 1) * _HPG)
        in_maps.append({
            "xT": xT[b],
            "wq": Qr[hs].reshape(_EG, _D).T.astype(f16),
            "wk": Kr[hs].reshape(_EG, _D).T.astype(f16),
            "wv": Vr[hs].reshape(_EG, _D).T.astype(f16),
            "wo": O_w[:, hg * _EG:(hg + 1) * _EG].T.astype(f16),
            "cs": cs, "sn": sn, "tri": tri, "perm": permT,
            "vones": np.ones((128, _HD), bf16),
        })
    return in_maps


def run_on_hw(in_maps, trace=False, **kw):
    from concourse.bass_utils import run_bass_kernel_spmd
    nc = get_nc()
    return run_bass_kernel_spmd(nc, in_maps, core_ids=list(range(_NCORES)),
                                trace=trace, **kw)


def kernel(x, token_positions, Q, K, V, O_w):
    in_maps = make_in_maps(x, token_positions, Q, K, V, O_w)
    res = run_on_hw(in_maps)
    out = np.zeros((_B, _L, _D), dtype=np.float32)
    for core in range(_NCORES):
        out[core // 4] += np.asarray(res.results[core]["y"],
                                     dtype=np.float32)
    return out
